# revision 33
# baseline (speedup 1.0000x reference)
"""DGCN aggregation kernel for Trainium2 (8 NeuronCores, graph-parallel).

Math (per edge type t):
    xn     = (x - mu) / sigma                      (feature-wise, ddof=1)
    deg_t  = segsum(|ea_t|, dst) + 1
    S'_t[d, s] = sum_{e:(s->d)} dis[s] |ea| dis[d]   (+ 1/deg on the diagonal)
    h1_t   = relu(S'_t xn W1_t + b1_t)
    out_t  = relu(S'_t h1_t W2_t + b2_t)
    out    = concat_t(out_t) reshaped to (B*NN, S, 3*D2)

Device mapping: edges (+ implicit self loops) are sorted by dst; the
scatter-add is a one-hot matmul per 128-slot batch (segment-sum by dst),
sharded across 8 cores by contiguous 4096-node dst ranges.  Per-slot operand
rows (xn rows for layer 1; norm-scaled g = h1 W2 rows for layer 2, by src)
are staged by the host in slot order, so the device only runs sequential
streaming DMA + fp16 matmuls with fp32 PSUM accumulation — no on-device
gather (SWDGE descriptor generation at ~8 ns/row dominates otherwise).

Layer 1 packs slots into 16-dst-node groups padded to 384 slots (3 batches)
and software-pipelines the one-hot phase of tile i+1 against the dense
phase of tile i.  Layer 2 is pure DMA-bandwidth-bound, so its slots are
split main/overflow to cut padding: the first 256 slots of each group go to
the main stream (16-wide 0/1 dst mask, norms pre-folded into the g rows);
group tails go to a per-tile overflow stream with a 128-wide dst mask.
"""

import numpy as np

import concourse.bacc as bacc
import concourse.mybir as mybir
import concourse.tile as tile
from concourse.bass_utils import run_bass_kernel_spmd

F32 = mybir.dt.float32
F16 = mybir.dt.float16

# Problem constants (hardcoded per the harness contract).
N = 32768          # nodes = B*S*NN = 4*16*512
E = 524288         # edges
F_IN, D1, D2 = 128, 256, 128
NT = 3             # edge types
BATCH, SEQ, NNODE = 4, 16, 512
GW = NT * D2       # g row width = 384

NCORES = 8
NPC = N // NCORES          # nodes per core = 4096
GROUP = 16                 # dst nodes per one-hot group
BPG = 3                    # 128-edge batches per group (layer-1 padding)
SLOTS_PG = BPG * 128       # padded edge slots per group = 384
GROUPS_PC = NPC // GROUP   # 256 groups per core
BATCHES_PC = GROUPS_PC * BPG          # 768 batches per core (layer 1)
SLOTS_PC = GROUPS_PC * SLOTS_PG       # 98304 edge slots per core (layer 1)
TILES_PC = NPC // 128      # 32 dst tiles per core
BPT = BPG * 8              # layer-1 batches per dst tile = 24
W_OH = NT * GROUP          # layer-1 one-hot width = 48

# Layer-2 main/overflow split
MAIN_PG = 256                        # main slots per group (2 batches)
MB_PT = (MAIN_PG // 128) * 8         # main batches per tile = 16
MAINB_PC = TILES_PC * MB_PT          # main batches per core = 512
OVF_SLOTS = 384                      # overflow slots per tile (3 batches)
OB_PT = OVF_SLOTS // 128             # overflow batches per tile = 3
OVFB_PC = TILES_PC * OB_PT           # overflow batches per core = 96

# Set by test.py for profiling runs; grading runs keep this off.
TRACE = False
LAST_TIMING = {}

_NC_CACHE = {}


def _build_l1():
    nc = bacc.Bacc("TRN2", target_bir_lowering=False, debug=False)
    # per-slot stream: [xn row (128) | dst-slot id (1) | norms (3)]
    SW = F_IN + 4
    xeoh = nc.dram_tensor(
        "xeoh", [128, BATCHES_PC, SW], F16, kind="ExternalInput")
    iota1 = nc.dram_tensor("iota1", [128, BPT, GROUP], F16, kind="ExternalInput")
    w1 = nc.dram_tensor("w1", [F_IN, NT, D1], F16, kind="ExternalInput")
    b1 = nc.dram_tensor("b1", [128, NT * 2], F32, kind="ExternalInput")
    w2 = nc.dram_tensor("w2", [128, NT, 2, D2], F16, kind="ExternalInput")
    g16 = nc.dram_tensor("g16", [NPC, GW], F16, kind="ExternalOutput")

    with tile.TileContext(nc) as tc:
        with (
            tc.tile_pool(name="const", bufs=1) as cpool,
            tc.tile_pool(name="sb", bufs=4) as sb,
            tc.tile_pool(name="ohp", bufs=3) as ohp,
            tc.tile_pool(name="mt", bufs=4) as mt,
            tc.tile_pool(name="hh", bufs=6) as hh,
            tc.tile_pool(name="sbo", bufs=3) as sbo,
            tc.tile_pool(name="ps", bufs=2, space="PSUM") as ps,
            tc.tile_pool(name="ps2", bufs=3, space="PSUM") as ps2,
            tc.tile_pool(name="ps3", bufs=2, space="PSUM") as ps3,
        ):
            w1_t = cpool.tile([F_IN, NT, D1], F16)
            nc.sync.dma_start(out=w1_t[:], in_=w1[:, :, :])
            b1_t = cpool.tile([128, NT * 2], F32)
            nc.sync.dma_start(out=b1_t[:], in_=b1[:, :])
            w2_t = cpool.tile([128, NT, 2, D2], F16)
            nc.sync.dma_start(out=w2_t[:], in_=w2[:, :, :, :])
            io1_t = cpool.tile([128, BPT, GROUP], F16)
            nc.sync.dma_start(out=io1_t[:], in_=iota1[:, :, :])

            def phase_a(ti):
                """stream + on-DVE one-hot build + aggregation + cast"""
                xg = sb.tile([128, BPT, SW], F16, tag="xg")
                nc.sync.dma_start(
                    out=xg[:], in_=xeoh[:, ti * BPT:(ti + 1) * BPT, :])
                # build the 48-wide norm one-hot from sid + norms
                mk = ohp.tile([128, BPT, GROUP], F16, tag="mk")
                nc.vector.tensor_tensor(
                    mk[:], io1_t[:],
                    xg[:, :, F_IN:F_IN + 1].to_broadcast([128, BPT, GROUP]),
                    mybir.AluOpType.is_equal)
                oh_t = ohp.tile([128, BPT, W_OH], F16, tag="oh")
                for t in range(NT):
                    # on gpsimd: DVE is on the critical cast path already
                    nc.gpsimd.tensor_tensor(
                        oh_t[:, :, t * GROUP:(t + 1) * GROUP], mk[:],
                        xg[:, :, F_IN + 1 + t:F_IN + 2 + t].to_broadcast(
                            [128, BPT, GROUP]),
                        mybir.AluOpType.mult)
                # m1T[f, (group, type, slot)] accumulated per 16-node group
                m1_ps = ps.tile([128, 8 * W_OH], F32, space="PSUM", tag="m1")
                for g8 in range(8):
                    for b in range(BPG):
                        bl = g8 * BPG + b
                        nc.tensor.matmul(
                            out=m1_ps[:, g8 * W_OH:(g8 + 1) * W_OH],
                            lhsT=xg[:, bl, :F_IN],
                            rhs=oh_t[:, bl, :],
                            start=(b == 0), stop=(b == BPG - 1),
                        )
                # de-interleave all types: [p, t, (g s)] = [128, 3, 128]
                m1t = mt.tile([128, NT, 128], F16, tag="m1t")
                nc.vector.tensor_copy(
                    out=m1t[:],
                    in_=m1_ps[:].rearrange("p (g t s) -> p t g s", g=8, t=NT))
                return m1t

            def phase_b(ti, m1t):
                """dense h1 = relu(m1 W1 + b1); g = h1 W2; writeback.
                All h1 matmuls are issued before any g matmul so the relus
                complete in the shadow of other PE work."""
                h1ts = []
                for t in range(NT):
                    h1_ps = ps2.tile([128, D1], F32, space="PSUM", tag="h1")
                    h1t = hh.tile([128, D1], F16, tag="h1t")
                    for c in range(2):
                        nc.tensor.matmul(
                            out=h1_ps[:, c * 128:(c + 1) * 128],
                            lhsT=w1_t[:, t, c * 128:(c + 1) * 128],
                            rhs=m1t[:, t, :],
                            start=True, stop=True,
                        )
                        nc.scalar.activation(
                            out=h1t[:, c * 128:(c + 1) * 128],
                            in_=h1_ps[:, c * 128:(c + 1) * 128],
                            func=mybir.ActivationFunctionType.Relu,
                            bias=b1_t[:, t * 2 + c: t * 2 + c + 1], scale=1.0,
                        )
                    h1ts.append(h1t)
                g_sb = sbo.tile([128, GW], F16, tag="gout")
                g_ps = ps3.tile([128, GW], F32, space="PSUM", tag="g")
                for t in range(NT):
                    nc.tensor.matmul(
                        out=g_ps[:, t * D2:(t + 1) * D2],
                        lhsT=h1ts[t][:, :128], rhs=w2_t[:, t, 0, :],
                        start=True, stop=False,
                    )
                    nc.tensor.matmul(
                        out=g_ps[:, t * D2:(t + 1) * D2],
                        lhsT=h1ts[t][:, 128:], rhs=w2_t[:, t, 1, :],
                        start=False, stop=True,
                    )
                nc.vector.tensor_copy(out=g_sb[:], in_=g_ps[:])
                nc.scalar.dma_start(
                    out=g16[ti * 128:(ti + 1) * 128, :], in_=g_sb[:])

            pending = []
            for ti in range(TILES_PC):
                m1t = phase_a(ti)
                pending.append((ti, m1t))
                if len(pending) > 2:
                    phase_b(*pending.pop(0))
            for p in pending:
                phase_b(*p)
    nc.compile()
    return nc


def _build_l2():
    nc = bacc.Bacc("TRN2", target_bir_lowering=False, debug=False)
    # main stream: [norm-scaled g rows (3*128) | dst-slot id (1)]
    gem = nc.dram_tensor(
        "gem", [128, MAINB_PC, GW + 1], F16, kind="ExternalInput")
    # overflow stream: [norm-scaled g rows (3*128) | dst-in-tile id (1)]
    gov = nc.dram_tensor(
        "gov", [128, OVFB_PC, GW + 1], F16, kind="ExternalInput")
    iota2m = nc.dram_tensor(
        "iota2m", [128, MB_PT, GROUP], F16, kind="ExternalInput")
    iota2o = nc.dram_tensor(
        "iota2o", [128, OB_PT, 128], F16, kind="ExternalInput")
    b2 = nc.dram_tensor("b2", [128, NT], F32, kind="ExternalInput")
    out2 = nc.dram_tensor(
        "out2", [D2, TILES_PC, NT, 128], F16, kind="ExternalOutput")

    with tile.TileContext(nc) as tc:
        with (
            tc.tile_pool(name="const", bufs=1) as cpool,
            tc.tile_pool(name="sb", bufs=4) as sb,
            tc.tile_pool(name="mkp", bufs=3) as mkp,
            tc.tile_pool(name="sbo", bufs=3) as sbo,
            tc.tile_pool(name="ps", bufs=4, space="PSUM") as ps,
        ):
            b2_t = cpool.tile([128, NT], F32)
            nc.sync.dma_start(out=b2_t[:], in_=b2[:, :])
            io2m_t = cpool.tile([128, MB_PT, GROUP], F16)
            nc.sync.dma_start(out=io2m_t[:], in_=iota2m[:, :, :])
            io2o_t = cpool.tile([128, OB_PT, 128], F16)
            nc.sync.dma_start(out=io2o_t[:], in_=iota2o[:, :, :])

            for ti in range(TILES_PC):
                gg = sb.tile([128, MB_PT, GW + 1], F16, tag="gg")
                nc.sync.dma_start(
                    out=gg[:], in_=gem[:, ti * MB_PT:(ti + 1) * MB_PT, :])
                go = sb.tile([128, OB_PT, GW + 1], F16, tag="go")
                nc.sync.dma_start(
                    out=go[:], in_=gov[:, ti * OB_PT:(ti + 1) * OB_PT, :])
                mkm = mkp.tile([128, MB_PT, GROUP], F16, tag="mkm")
                nc.vector.tensor_tensor(
                    mkm[:], io2m_t[:],
                    gg[:, :, GW:GW + 1].to_broadcast([128, MB_PT, GROUP]),
                    mybir.AluOpType.is_equal)
                mko = mkp.tile([128, OB_PT, 128], F16, tag="mko")
                nc.vector.tensor_tensor(
                    mko[:], io2o_t[:],
                    go[:, :, GW:GW + 1].to_broadcast([128, OB_PT, 128]),
                    mybir.AluOpType.is_equal)
                o_sb = sbo.tile([128, NT, 128], F16, tag="osb")
                for t in range(NT):
                    # m2T_t [d2, node-within-tile]: main windows + overflow
                    m2_ps = ps.tile([128, 128], F32, space="PSUM", tag="m2")
                    for g8 in range(8):
                        for b in range(2):
                            bl = g8 * 2 + b
                            nc.tensor.matmul(
                                out=m2_ps[:, g8 * GROUP:(g8 + 1) * GROUP],
                                lhsT=gg[:, bl, t * D2:(t + 1) * D2],
                                rhs=mkm[:, bl, :],
                                start=(b == 0), stop=(b == 1),
                            )
                    m2o_ps = ps.tile([128, 128], F32, space="PSUM", tag="m2o")
                    for b in range(OB_PT):
                        nc.tensor.matmul(
                            out=m2o_ps[:],
                            lhsT=go[:, b, t * D2:(t + 1) * D2],
                            rhs=mko[:, b, :],
                            start=(b == 0), stop=(b == OB_PT - 1),
                        )
                    o2_sb = sbo.tile([128, 128], F32, tag="o2sb")
                    nc.vector.tensor_copy(out=o2_sb[:], in_=m2o_ps[:])
                    s_sb = sbo.tile([128, 128], F32, tag="ssb")
                    nc.vector.tensor_tensor(
                        s_sb[:], m2_ps[:], o2_sb[:], mybir.AluOpType.add)
                    nc.scalar.activation(
                        out=o_sb[:, t, :], in_=s_sb[:],
                        func=mybir.ActivationFunctionType.Relu,
                        bias=b2_t[:, t:t + 1], scale=1.0,
                    )
                nc.scalar.dma_start(out=out2[:, ti, :, :], in_=o_sb[:])
    nc.compile()
    return nc


def _host_prep(x, edge_attr, edge_index):
    """Sort/shard/pad edges, normalize x, stage the layer-1 stream and the
    layer-2 slot assignment (main/overflow)."""
    src = np.asarray(edge_index[0], np.int64)
    dst = np.asarray(edge_index[1], np.int64)
    ew = np.abs(np.asarray(edge_attr, np.float32))          # [E, 3]

    deg = np.empty((N, NT), np.float32)
    for t in range(NT):
        deg[:, t] = np.bincount(dst, weights=ew[:, t], minlength=N)
    deg += 1.0
    dis = 1.0 / np.sqrt(deg)

    norm = dis[src] * ew * dis[dst]                          # [E, 3]
    src_all = np.concatenate([src, np.arange(N)])
    dst_all = np.concatenate([dst, np.arange(N)])
    norm_all = np.concatenate([norm, 1.0 / deg]).astype(np.float32)

    order = np.argsort(dst_all, kind="stable")
    sa = src_all[order]
    da = dst_all[order]
    na = norm_all[order].astype(np.float16)

    gid = da >> 4                                            # 16-node group id
    counts = np.bincount(gid, minlength=N // GROUP)
    assert counts.max() <= SLOTS_PG, (
        f"group overflow: {counts.max()} > {SLOTS_PG}")
    gstart = np.zeros(N // GROUP + 1, np.int64)
    np.cumsum(counts, out=gstart[1:])
    rank = np.arange(da.size) - gstart[gid]

    # ---- layer-1 slot layout: 384 padded slots per group -------------
    pos = gid * SLOTS_PG + rank
    n_slots = (N // GROUP) * SLOTS_PG
    src_pad = np.zeros(n_slots, np.int64)
    src_pad[pos] = sa
    slot = (da & (GROUP - 1)).astype(np.int64)
    sid1 = np.full(n_slots, 255.0, np.float16)
    sid1[pos] = slot
    na1 = np.zeros((n_slots, NT), np.float16)
    na1[pos] = na

    # ---- layer-2 slot layout: 256 main slots per group + overflow ----
    mm = rank < MAIN_PG
    pos_m = gid[mm] * MAIN_PG + rank[mm]
    n_main = (N // GROUP) * MAIN_PG
    src_m = np.zeros(n_main, np.int64)
    src_m[pos_m] = sa[mm]
    na_m = np.zeros((n_main, NT), np.float16)
    na_m[pos_m] = na[mm]
    sid_m = np.full(n_main, 255.0, np.float16)
    sid_m[pos_m] = slot[mm]

    ov = ~mm
    tile_e = da[ov] >> 7                                     # global dst tile
    cnt_o = np.bincount(tile_e, minlength=N // 128)
    assert cnt_o.max() <= OVF_SLOTS, (
        f"tile overflow: {cnt_o.max()} > {OVF_SLOTS}")
    st_o = np.zeros(N // 128 + 1, np.int64)
    np.cumsum(cnt_o, out=st_o[1:])
    r2 = np.arange(tile_e.size) - st_o[tile_e]
    pos_o = tile_e * OVF_SLOTS + r2
    n_ovf = (N // 128) * OVF_SLOTS
    src_o = np.zeros(n_ovf, np.int64)
    src_o[pos_o] = sa[ov]
    na_o = np.zeros((n_ovf, NT), np.float16)
    na_o[pos_o] = na[ov]
    sid_o = np.full(n_ovf, 255.0, np.float16)
    sid_o[pos_o] = da[ov] & 127

    # normalize x on the host (fp16 device math, fp32 accumulation)
    mu = x.mean(axis=0)
    sg = x.std(axis=0, ddof=1)
    xn16 = ((x - mu[None, :]) / sg[None, :]).astype(np.float16)

    def pb(a, nb):
        """[nb*128, ...] -> [128, nb, ...] (partition = slot % 128)"""
        return a.reshape((nb, 128) + a.shape[1:]).swapaxes(0, 1)

    per_core = []
    for k in range(NCORES):
        # [p, b] layout everywhere: partition = slot % 128, batch = slot // 128
        s1 = slice(k * SLOTS_PC, (k + 1) * SLOTS_PC)
        idx1 = pb(src_pad[s1], BATCHES_PC)
        xeoh = np.empty((128, BATCHES_PC, F_IN + 4), np.float16)
        np.take(xn16, idx1, axis=0, out=xeoh[:, :, :F_IN])
        xeoh[:, :, F_IN] = pb(sid1[s1], BATCHES_PC)
        xeoh[:, :, F_IN + 1:] = pb(na1[s1], BATCHES_PC)

        s_m = slice(k * MAINB_PC * 128, (k + 1) * MAINB_PC * 128)
        s_o = slice(k * OVFB_PC * 128, (k + 1) * OVFB_PC * 128)
        per_core.append((
            xeoh,
            pb(src_m[s_m], MAINB_PC), pb(na_m[s_m], MAINB_PC),
            pb(sid_m[s_m], MAINB_PC),
            pb(src_o[s_o], OVFB_PC), pb(na_o[s_o], OVFB_PC),
            pb(sid_o[s_o], OVFB_PC),
        ))
    return per_core


def _stage_l2(g_full, idx_pb, na_pb, sid_pb, nb):
    """Build a layer-2 stream tensor [128, nb, GW + 1]: norm-scaled
    gathered g rows followed by the dst slot id."""
    out = np.empty((128, nb, GW + 1), np.float16)
    np.take(g_full, idx_pb, axis=0, out=out[:, :, :GW])
    for t in range(NT):
        out[:, :, t * D2:(t + 1) * D2] *= na_pb[:, :, t:t + 1]
    out[:, :, GW] = sid_pb
    return out


def kernel(x, edge_attr, W1, b1, W2, b2, edge_index, batch_size, seq_len,
           n_nodes):
    x = np.asarray(x, np.float32)
    edge_attr = np.asarray(edge_attr, np.float32)
    W1 = np.asarray(W1, np.float32)
    b1 = np.asarray(b1, np.float32)
    W2 = np.asarray(W2, np.float32)
    b2 = np.asarray(b2, np.float32)
    edge_index = np.asarray(edge_index)
    assert x.shape == (N, F_IN) and edge_index.shape == (2, E)

    per_core = _host_prep(x, edge_attr, edge_index)

    # ---- launch 1 ----
    if "l1" not in _NC_CACHE:
        _NC_CACHE["l1"] = _build_l1()
    nc1 = _NC_CACHE["l1"]

    w1_in = np.ascontiguousarray(W1.transpose(1, 0, 2)).astype(np.float16)
    b1_in = np.ascontiguousarray(
        b1.reshape(NT, 2, 128).transpose(2, 0, 1).reshape(128, NT * 2))
    w2_in = np.ascontiguousarray(
        W2.reshape(NT, 2, 128, D2).transpose(2, 0, 1, 3)).astype(np.float16)

    iota1_in = np.broadcast_to(
        np.arange(GROUP, dtype=np.float16), (128, BPT, GROUP)).copy()
    in_maps1 = []
    for k in range(NCORES):
        in_maps1.append({
            "xeoh": per_core[k][0], "iota1": iota1_in,
            "w1": w1_in, "b1": b1_in, "w2": w2_in,
        })
    res1 = run_bass_kernel_spmd(
        nc1, in_maps1, core_ids=list(range(NCORES)), trace=TRACE)
    if TRACE:
        LAST_TIMING["l1_ns"] = res1.exec_time_ns

    g_full = np.concatenate(
        [res1.results[k]["g16"] for k in range(NCORES)], axis=0)  # [N, 384] f16

    # ---- launch 2 ----
    if "l2" not in _NC_CACHE:
        _NC_CACHE["l2"] = _build_l2()
    nc2 = _NC_CACHE["l2"]

    b2_in = np.ascontiguousarray(b2.T)                            # [128, 3]
    iota2m_in = np.broadcast_to(
        np.arange(GROUP, dtype=np.float16), (128, MB_PT, GROUP)).copy()
    iota2o_in = np.broadcast_to(
        np.arange(128, dtype=np.float16), (128, OB_PT, 128)).copy()
    in_maps2 = []
    for k in range(NCORES):
        _, idx_m, na_m_pb, sid_m_pb, idx_o, na_o_pb, sid_o_pb = per_core[k]
        in_maps2.append({
            "gem": _stage_l2(g_full, idx_m, na_m_pb, sid_m_pb, MAINB_PC),
            "gov": _stage_l2(g_full, idx_o, na_o_pb, sid_o_pb, OVFB_PC),
            "iota2m": iota2m_in, "iota2o": iota2o_in,
            "b2": b2_in,
        })
    res2 = run_bass_kernel_spmd(
        nc2, in_maps2, core_ids=list(range(NCORES)), trace=TRACE)
    if TRACE:
        LAST_TIMING["l2_ns"] = res2.exec_time_ns

    # per-core out2 [D2, TILES, NT, 128] -> [NT, D2, NPC]; concat cores
    m2t = np.concatenate(
        [res2.results[k]["out2"].transpose(2, 0, 1, 3).reshape(NT, D2, NPC)
         for k in range(NCORES)], axis=2)                          # [3,128,N] f16

    # [3, 128, (b, s, nn)] -> out[(b, nn), s, (t, d)]
    out = m2t.astype(np.float32).reshape(NT, D2, BATCH, SEQ, NNODE)
    out = out.transpose(2, 4, 3, 0, 1)
    out = np.ascontiguousarray(
        out.reshape(BATCH * NNODE, SEQ, NT * D2), dtype=np.float32)
    return out


# revision 34
# speedup vs baseline: 1.0162x; 1.0162x over previous
"""DGCN aggregation kernel for Trainium2 (8 NeuronCores, graph-parallel).

Math (per edge type t):
    xn     = (x - mu) / sigma                      (feature-wise, ddof=1)
    deg_t  = segsum(|ea_t|, dst) + 1
    S'_t[d, s] = sum_{e:(s->d)} dis[s] |ea| dis[d]   (+ 1/deg on the diagonal)
    h1_t   = relu(S'_t xn W1_t + b1_t)
    out_t  = relu(S'_t h1_t W2_t + b2_t)
    out    = concat_t(out_t) reshaped to (B*NN, S, 3*D2)

Device mapping: edges (+ implicit self loops) are sorted by dst; the
scatter-add is a one-hot matmul per 128-slot batch (segment-sum by dst),
sharded across 8 cores by contiguous 4096-node dst ranges.  Per-slot operand
rows (xn rows for layer 1; norm-scaled g = h1 W2 rows for layer 2, by src)
are staged by the host in slot order, so the device only runs sequential
streaming DMA + fp16 matmuls with fp32 PSUM accumulation — no on-device
gather (SWDGE descriptor generation at ~8 ns/row dominates otherwise).

Layer 1 packs slots into 16-dst-node groups padded to 384 slots (3 batches)
and software-pipelines the one-hot phase of tile i+1 against the dense
phase of tile i.  Layer 2 is pure DMA-bandwidth-bound, so its slots are
split main/overflow to cut padding: the first 256 slots of each group go to
the main stream (16-wide 0/1 dst mask, norms pre-folded into the g rows);
group tails go to a per-tile overflow stream with a 128-wide dst mask.
"""

import numpy as np

import concourse.bacc as bacc
import concourse.mybir as mybir
import concourse.tile as tile
from concourse.bass_utils import run_bass_kernel_spmd

F32 = mybir.dt.float32
F16 = mybir.dt.float16

# Problem constants (hardcoded per the harness contract).
N = 32768          # nodes = B*S*NN = 4*16*512
E = 524288         # edges
F_IN, D1, D2 = 128, 256, 128
NT = 3             # edge types
BATCH, SEQ, NNODE = 4, 16, 512
GW = NT * D2       # g row width = 384

NCORES = 8
NPC = N // NCORES          # nodes per core = 4096
GROUP = 16                 # dst nodes per one-hot group
BPG = 3                    # 128-edge batches per group (layer-1 padding)
SLOTS_PG = BPG * 128       # padded edge slots per group = 384
GROUPS_PC = NPC // GROUP   # 256 groups per core
BATCHES_PC = GROUPS_PC * BPG          # 768 batches per core (layer 1)
SLOTS_PC = GROUPS_PC * SLOTS_PG       # 98304 edge slots per core (layer 1)
TILES_PC = NPC // 128      # 32 dst tiles per core
BPT = BPG * 8              # layer-1 batches per dst tile = 24
W_OH = NT * GROUP          # layer-1 one-hot width = 48

# Layer-2 main/overflow split
MAIN_PG = 256                        # main slots per group (2 batches)
MB_PT = (MAIN_PG // 128) * 8         # main batches per tile = 16
MAINB_PC = TILES_PC * MB_PT          # main batches per core = 512
OVF_SLOTS = 384                      # overflow slots per tile (3 batches)
OB_PT = OVF_SLOTS // 128             # overflow batches per tile = 3
OVFB_PC = TILES_PC * OB_PT           # overflow batches per core = 96

# Set by test.py for profiling runs; grading runs keep this off.
TRACE = False
LAST_TIMING = {}

_NC_CACHE = {}


def _build_l1():
    nc = bacc.Bacc("TRN2", target_bir_lowering=False, debug=False)
    # per-slot stream: [xn row (128) | dst-slot id (1) | norms (3)]
    SW = F_IN + 4
    xeoh = nc.dram_tensor(
        "xeoh", [128, BATCHES_PC, SW], F16, kind="ExternalInput")
    iota1 = nc.dram_tensor("iota1", [128, BPT, GROUP], F16, kind="ExternalInput")
    w1 = nc.dram_tensor("w1", [F_IN, NT, D1], F16, kind="ExternalInput")
    b1 = nc.dram_tensor("b1", [128, NT * 2], F32, kind="ExternalInput")
    w2 = nc.dram_tensor("w2", [128, NT, 2, D2], F16, kind="ExternalInput")
    g16 = nc.dram_tensor("g16", [NPC, GW], F16, kind="ExternalOutput")

    with tile.TileContext(nc) as tc:
        with (
            tc.tile_pool(name="const", bufs=1) as cpool,
            tc.tile_pool(name="sb", bufs=4) as sb,
            tc.tile_pool(name="ohp", bufs=3) as ohp,
            tc.tile_pool(name="mt", bufs=4) as mt,
            tc.tile_pool(name="hh", bufs=6) as hh,
            tc.tile_pool(name="sbo", bufs=3) as sbo,
            tc.tile_pool(name="ps", bufs=2, space="PSUM") as ps,
            tc.tile_pool(name="ps2", bufs=3, space="PSUM") as ps2,
            tc.tile_pool(name="ps3", bufs=2, space="PSUM") as ps3,
        ):
            w1_t = cpool.tile([F_IN, NT, D1], F16)
            nc.sync.dma_start(out=w1_t[:], in_=w1[:, :, :])
            b1_t = cpool.tile([128, NT * 2], F32)
            nc.sync.dma_start(out=b1_t[:], in_=b1[:, :])
            w2_t = cpool.tile([128, NT, 2, D2], F16)
            nc.sync.dma_start(out=w2_t[:], in_=w2[:, :, :, :])
            io1_t = cpool.tile([128, BPT, GROUP], F16)
            nc.sync.dma_start(out=io1_t[:], in_=iota1[:, :, :])

            def phase_a(ti):
                """stream + on-DVE one-hot build + aggregation + cast"""
                xg = sb.tile([128, BPT, SW], F16, tag="xg")
                nc.sync.dma_start(
                    out=xg[:], in_=xeoh[:, ti * BPT:(ti + 1) * BPT, :])
                # build the 48-wide norm one-hot from sid + norms
                mk = ohp.tile([128, BPT, GROUP], F16, tag="mk")
                nc.vector.tensor_tensor(
                    mk[:], io1_t[:],
                    xg[:, :, F_IN:F_IN + 1].to_broadcast([128, BPT, GROUP]),
                    mybir.AluOpType.is_equal)
                oh_t = ohp.tile([128, BPT, W_OH], F16, tag="oh")
                for t in range(NT):
                    nc.vector.tensor_tensor(
                        oh_t[:, :, t * GROUP:(t + 1) * GROUP], mk[:],
                        xg[:, :, F_IN + 1 + t:F_IN + 2 + t].to_broadcast(
                            [128, BPT, GROUP]),
                        mybir.AluOpType.mult)
                # m1T[f, (group, type, slot)] accumulated per 16-node group
                m1_ps = ps.tile([128, 8 * W_OH], F32, space="PSUM", tag="m1")
                for g8 in range(8):
                    for b in range(BPG):
                        bl = g8 * BPG + b
                        nc.tensor.matmul(
                            out=m1_ps[:, g8 * W_OH:(g8 + 1) * W_OH],
                            lhsT=xg[:, bl, :F_IN],
                            rhs=oh_t[:, bl, :],
                            start=(b == 0), stop=(b == BPG - 1),
                        )
                # de-interleave all types: [p, t, (g s)] = [128, 3, 128]
                m1t = mt.tile([128, NT, 128], F16, tag="m1t")
                nc.vector.tensor_copy(
                    out=m1t[:],
                    in_=m1_ps[:].rearrange("p (g t s) -> p t g s", g=8, t=NT))
                return m1t

            def phase_b(ti, m1t):
                """dense h1 = relu(m1 W1 + b1); g = h1 W2; writeback.
                All h1 matmuls are issued before any g matmul so the relus
                complete in the shadow of other PE work."""
                h1ts = []
                for t in range(NT):
                    h1_ps = ps2.tile([128, D1], F32, space="PSUM", tag="h1")
                    h1t = hh.tile([128, D1], F16, tag="h1t")
                    for c in range(2):
                        nc.tensor.matmul(
                            out=h1_ps[:, c * 128:(c + 1) * 128],
                            lhsT=w1_t[:, t, c * 128:(c + 1) * 128],
                            rhs=m1t[:, t, :],
                            start=True, stop=True,
                        )
                        nc.scalar.activation(
                            out=h1t[:, c * 128:(c + 1) * 128],
                            in_=h1_ps[:, c * 128:(c + 1) * 128],
                            func=mybir.ActivationFunctionType.Relu,
                            bias=b1_t[:, t * 2 + c: t * 2 + c + 1], scale=1.0,
                        )
                    h1ts.append(h1t)
                g_sb = sbo.tile([128, GW], F16, tag="gout")
                g_ps = ps3.tile([128, GW], F32, space="PSUM", tag="g")
                for t in range(NT):
                    nc.tensor.matmul(
                        out=g_ps[:, t * D2:(t + 1) * D2],
                        lhsT=h1ts[t][:, :128], rhs=w2_t[:, t, 0, :],
                        start=True, stop=False,
                    )
                    nc.tensor.matmul(
                        out=g_ps[:, t * D2:(t + 1) * D2],
                        lhsT=h1ts[t][:, 128:], rhs=w2_t[:, t, 1, :],
                        start=False, stop=True,
                    )
                nc.vector.tensor_copy(out=g_sb[:], in_=g_ps[:])
                nc.scalar.dma_start(
                    out=g16[ti * 128:(ti + 1) * 128, :], in_=g_sb[:])

            pending = []
            for ti in range(TILES_PC):
                m1t = phase_a(ti)
                pending.append((ti, m1t))
                if len(pending) > 2:
                    phase_b(*pending.pop(0))
            for p in pending:
                phase_b(*p)
    nc.compile()
    return nc


def _build_l2():
    nc = bacc.Bacc("TRN2", target_bir_lowering=False, debug=False)
    # main stream: [norm-scaled g rows (3*128) | dst-slot id (1)]
    gem = nc.dram_tensor(
        "gem", [128, MAINB_PC, GW + 1], F16, kind="ExternalInput")
    # overflow stream: [norm-scaled g rows (3*128) | dst-in-tile id (1)]
    gov = nc.dram_tensor(
        "gov", [128, OVFB_PC, GW + 1], F16, kind="ExternalInput")
    iota2m = nc.dram_tensor(
        "iota2m", [128, MB_PT, GROUP], F16, kind="ExternalInput")
    iota2o = nc.dram_tensor(
        "iota2o", [128, OB_PT, 128], F16, kind="ExternalInput")
    b2 = nc.dram_tensor("b2", [128, NT], F32, kind="ExternalInput")
    out2 = nc.dram_tensor(
        "out2", [D2, TILES_PC, NT, 128], F16, kind="ExternalOutput")

    with tile.TileContext(nc) as tc:
        with (
            tc.tile_pool(name="const", bufs=1) as cpool,
            tc.tile_pool(name="sb", bufs=4) as sb,
            tc.tile_pool(name="mkp", bufs=3) as mkp,
            tc.tile_pool(name="sbo", bufs=3) as sbo,
            tc.tile_pool(name="ps", bufs=4, space="PSUM") as ps,
        ):
            b2_t = cpool.tile([128, NT], F32)
            nc.sync.dma_start(out=b2_t[:], in_=b2[:, :])
            io2m_t = cpool.tile([128, MB_PT, GROUP], F16)
            nc.sync.dma_start(out=io2m_t[:], in_=iota2m[:, :, :])
            io2o_t = cpool.tile([128, OB_PT, 128], F16)
            nc.sync.dma_start(out=io2o_t[:], in_=iota2o[:, :, :])

            for ti in range(TILES_PC):
                gg = sb.tile([128, MB_PT, GW + 1], F16, tag="gg")
                nc.sync.dma_start(
                    out=gg[:], in_=gem[:, ti * MB_PT:(ti + 1) * MB_PT, :])
                go = sb.tile([128, OB_PT, GW + 1], F16, tag="go")
                nc.sync.dma_start(
                    out=go[:], in_=gov[:, ti * OB_PT:(ti + 1) * OB_PT, :])
                mkm = mkp.tile([128, MB_PT, GROUP], F16, tag="mkm")
                nc.vector.tensor_tensor(
                    mkm[:], io2m_t[:],
                    gg[:, :, GW:GW + 1].to_broadcast([128, MB_PT, GROUP]),
                    mybir.AluOpType.is_equal)
                mko = mkp.tile([128, OB_PT, 128], F16, tag="mko")
                nc.vector.tensor_tensor(
                    mko[:], io2o_t[:],
                    go[:, :, GW:GW + 1].to_broadcast([128, OB_PT, 128]),
                    mybir.AluOpType.is_equal)
                o_sb = sbo.tile([128, NT, 128], F16, tag="osb")
                for t in range(NT):
                    # m2T_t [d2, node-within-tile]: main windows + overflow
                    m2_ps = ps.tile([128, 128], F32, space="PSUM", tag="m2")
                    for g8 in range(8):
                        for b in range(2):
                            bl = g8 * 2 + b
                            nc.tensor.matmul(
                                out=m2_ps[:, g8 * GROUP:(g8 + 1) * GROUP],
                                lhsT=gg[:, bl, t * D2:(t + 1) * D2],
                                rhs=mkm[:, bl, :],
                                start=(b == 0), stop=(b == 1),
                            )
                    m2o_ps = ps.tile([128, 128], F32, space="PSUM", tag="m2o")
                    for b in range(OB_PT):
                        nc.tensor.matmul(
                            out=m2o_ps[:],
                            lhsT=go[:, b, t * D2:(t + 1) * D2],
                            rhs=mko[:, b, :],
                            start=(b == 0), stop=(b == OB_PT - 1),
                        )
                    o2_sb = sbo.tile([128, 128], F32, tag="o2sb")
                    nc.vector.tensor_copy(out=o2_sb[:], in_=m2o_ps[:])
                    s_sb = sbo.tile([128, 128], F32, tag="ssb")
                    nc.vector.tensor_tensor(
                        s_sb[:], m2_ps[:], o2_sb[:], mybir.AluOpType.add)
                    nc.scalar.activation(
                        out=o_sb[:, t, :], in_=s_sb[:],
                        func=mybir.ActivationFunctionType.Relu,
                        bias=b2_t[:, t:t + 1], scale=1.0,
                    )
                nc.scalar.dma_start(out=out2[:, ti, :, :], in_=o_sb[:])
    nc.compile()
    return nc


def _host_prep(x, edge_attr, edge_index):
    """Sort/shard/pad edges, normalize x, stage the layer-1 stream and the
    layer-2 slot assignment (main/overflow)."""
    src = np.asarray(edge_index[0], np.int64)
    dst = np.asarray(edge_index[1], np.int64)
    ew = np.abs(np.asarray(edge_attr, np.float32))          # [E, 3]

    deg = np.empty((N, NT), np.float32)
    for t in range(NT):
        deg[:, t] = np.bincount(dst, weights=ew[:, t], minlength=N)
    deg += 1.0
    dis = 1.0 / np.sqrt(deg)

    norm = dis[src] * ew * dis[dst]                          # [E, 3]
    src_all = np.concatenate([src, np.arange(N)])
    dst_all = np.concatenate([dst, np.arange(N)])
    norm_all = np.concatenate([norm, 1.0 / deg]).astype(np.float32)

    order = np.argsort(dst_all, kind="stable")
    sa = src_all[order]
    da = dst_all[order]
    na = norm_all[order].astype(np.float16)

    gid = da >> 4                                            # 16-node group id
    counts = np.bincount(gid, minlength=N // GROUP)
    assert counts.max() <= SLOTS_PG, (
        f"group overflow: {counts.max()} > {SLOTS_PG}")
    gstart = np.zeros(N // GROUP + 1, np.int64)
    np.cumsum(counts, out=gstart[1:])
    rank = np.arange(da.size) - gstart[gid]

    # ---- layer-1 slot layout: 384 padded slots per group -------------
    pos = gid * SLOTS_PG + rank
    n_slots = (N // GROUP) * SLOTS_PG
    src_pad = np.zeros(n_slots, np.int64)
    src_pad[pos] = sa
    slot = (da & (GROUP - 1)).astype(np.int64)
    sid1 = np.full(n_slots, 255.0, np.float16)
    sid1[pos] = slot
    na1 = np.zeros((n_slots, NT), np.float16)
    na1[pos] = na

    # ---- layer-2 slot layout: 256 main slots per group + overflow ----
    mm = rank < MAIN_PG
    pos_m = gid[mm] * MAIN_PG + rank[mm]
    n_main = (N // GROUP) * MAIN_PG
    src_m = np.zeros(n_main, np.int64)
    src_m[pos_m] = sa[mm]
    na_m = np.zeros((n_main, NT), np.float16)
    na_m[pos_m] = na[mm]
    sid_m = np.full(n_main, 255.0, np.float16)
    sid_m[pos_m] = slot[mm]

    ov = ~mm
    tile_e = da[ov] >> 7                                     # global dst tile
    cnt_o = np.bincount(tile_e, minlength=N // 128)
    assert cnt_o.max() <= OVF_SLOTS, (
        f"tile overflow: {cnt_o.max()} > {OVF_SLOTS}")
    st_o = np.zeros(N // 128 + 1, np.int64)
    np.cumsum(cnt_o, out=st_o[1:])
    r2 = np.arange(tile_e.size) - st_o[tile_e]
    pos_o = tile_e * OVF_SLOTS + r2
    n_ovf = (N // 128) * OVF_SLOTS
    src_o = np.zeros(n_ovf, np.int64)
    src_o[pos_o] = sa[ov]
    na_o = np.zeros((n_ovf, NT), np.float16)
    na_o[pos_o] = na[ov]
    sid_o = np.full(n_ovf, 255.0, np.float16)
    sid_o[pos_o] = da[ov] & 127

    # normalize x on the host (fp16 device math, fp32 accumulation)
    mu = x.mean(axis=0)
    sg = x.std(axis=0, ddof=1)
    xn16 = ((x - mu[None, :]) / sg[None, :]).astype(np.float16)

    def pb(a, nb):
        """[nb*128, ...] -> [128, nb, ...] (partition = slot % 128)"""
        return a.reshape((nb, 128) + a.shape[1:]).swapaxes(0, 1)

    per_core = []
    for k in range(NCORES):
        # [p, b] layout everywhere: partition = slot % 128, batch = slot // 128
        s1 = slice(k * SLOTS_PC, (k + 1) * SLOTS_PC)
        idx1 = pb(src_pad[s1], BATCHES_PC)
        xeoh = np.empty((128, BATCHES_PC, F_IN + 4), np.float16)
        np.take(xn16, idx1, axis=0, out=xeoh[:, :, :F_IN])
        xeoh[:, :, F_IN] = pb(sid1[s1], BATCHES_PC)
        xeoh[:, :, F_IN + 1:] = pb(na1[s1], BATCHES_PC)

        s_m = slice(k * MAINB_PC * 128, (k + 1) * MAINB_PC * 128)
        s_o = slice(k * OVFB_PC * 128, (k + 1) * OVFB_PC * 128)
        per_core.append((
            xeoh,
            pb(src_m[s_m], MAINB_PC), pb(na_m[s_m], MAINB_PC),
            pb(sid_m[s_m], MAINB_PC),
            pb(src_o[s_o], OVFB_PC), pb(na_o[s_o], OVFB_PC),
            pb(sid_o[s_o], OVFB_PC),
        ))
    return per_core


def _stage_l2(g_full, idx_pb, na_pb, sid_pb, nb):
    """Build a layer-2 stream tensor [128, nb, GW + 1]: norm-scaled
    gathered g rows followed by the dst slot id."""
    out = np.empty((128, nb, GW + 1), np.float16)
    np.take(g_full, idx_pb, axis=0, out=out[:, :, :GW])
    for t in range(NT):
        out[:, :, t * D2:(t + 1) * D2] *= na_pb[:, :, t:t + 1]
    out[:, :, GW] = sid_pb
    return out


def kernel(x, edge_attr, W1, b1, W2, b2, edge_index, batch_size, seq_len,
           n_nodes):
    x = np.asarray(x, np.float32)
    edge_attr = np.asarray(edge_attr, np.float32)
    W1 = np.asarray(W1, np.float32)
    b1 = np.asarray(b1, np.float32)
    W2 = np.asarray(W2, np.float32)
    b2 = np.asarray(b2, np.float32)
    edge_index = np.asarray(edge_index)
    assert x.shape == (N, F_IN) and edge_index.shape == (2, E)

    per_core = _host_prep(x, edge_attr, edge_index)

    # ---- launch 1 ----
    if "l1" not in _NC_CACHE:
        _NC_CACHE["l1"] = _build_l1()
    nc1 = _NC_CACHE["l1"]

    w1_in = np.ascontiguousarray(W1.transpose(1, 0, 2)).astype(np.float16)
    b1_in = np.ascontiguousarray(
        b1.reshape(NT, 2, 128).transpose(2, 0, 1).reshape(128, NT * 2))
    w2_in = np.ascontiguousarray(
        W2.reshape(NT, 2, 128, D2).transpose(2, 0, 1, 3)).astype(np.float16)

    iota1_in = np.broadcast_to(
        np.arange(GROUP, dtype=np.float16), (128, BPT, GROUP)).copy()
    in_maps1 = []
    for k in range(NCORES):
        in_maps1.append({
            "xeoh": per_core[k][0], "iota1": iota1_in,
            "w1": w1_in, "b1": b1_in, "w2": w2_in,
        })
    res1 = run_bass_kernel_spmd(
        nc1, in_maps1, core_ids=list(range(NCORES)), trace=TRACE)
    if TRACE:
        LAST_TIMING["l1_ns"] = res1.exec_time_ns

    g_full = np.concatenate(
        [res1.results[k]["g16"] for k in range(NCORES)], axis=0)  # [N, 384] f16

    # ---- launch 2 ----
    if "l2" not in _NC_CACHE:
        _NC_CACHE["l2"] = _build_l2()
    nc2 = _NC_CACHE["l2"]

    b2_in = np.ascontiguousarray(b2.T)                            # [128, 3]
    iota2m_in = np.broadcast_to(
        np.arange(GROUP, dtype=np.float16), (128, MB_PT, GROUP)).copy()
    iota2o_in = np.broadcast_to(
        np.arange(128, dtype=np.float16), (128, OB_PT, 128)).copy()
    in_maps2 = []
    for k in range(NCORES):
        _, idx_m, na_m_pb, sid_m_pb, idx_o, na_o_pb, sid_o_pb = per_core[k]
        in_maps2.append({
            "gem": _stage_l2(g_full, idx_m, na_m_pb, sid_m_pb, MAINB_PC),
            "gov": _stage_l2(g_full, idx_o, na_o_pb, sid_o_pb, OVFB_PC),
            "iota2m": iota2m_in, "iota2o": iota2o_in,
            "b2": b2_in,
        })
    res2 = run_bass_kernel_spmd(
        nc2, in_maps2, core_ids=list(range(NCORES)), trace=TRACE)
    if TRACE:
        LAST_TIMING["l2_ns"] = res2.exec_time_ns

    # per-core out2 [D2, TILES, NT, 128] -> [NT, D2, NPC]; concat cores
    m2t = np.concatenate(
        [res2.results[k]["out2"].transpose(2, 0, 1, 3).reshape(NT, D2, NPC)
         for k in range(NCORES)], axis=2)                          # [3,128,N] f16

    # [3, 128, (b, s, nn)] -> out[(b, nn), s, (t, d)]
    out = m2t.astype(np.float32).reshape(NT, D2, BATCH, SEQ, NNODE)
    out = out.transpose(2, 4, 3, 0, 1)
    out = np.ascontiguousarray(
        out.reshape(BATCH * NNODE, SEQ, NT * D2), dtype=np.float32)
    return out


# revision 35
# speedup vs baseline: 1.0783x; 1.0611x over previous
"""DGCN aggregation kernel for Trainium2 (8 NeuronCores, graph-parallel).

Math (per edge type t):
    xn     = (x - mu) / sigma                      (feature-wise, ddof=1)
    deg_t  = segsum(|ea_t|, dst) + 1
    S'_t[d, s] = sum_{e:(s->d)} dis[s] |ea| dis[d]   (+ 1/deg on the diagonal)
    h1_t   = relu(S'_t xn W1_t + b1_t)
    out_t  = relu(S'_t h1_t W2_t + b2_t)
    out    = concat_t(out_t) reshaped to (B*NN, S, 3*D2)

Device mapping: edges (+ implicit self loops) are sorted by dst; the
scatter-add is a one-hot matmul per 128-slot batch (segment-sum by dst),
sharded across 8 cores by contiguous 4096-node dst ranges.  Per-slot operand
rows (xn rows for layer 1; norm-scaled g = h1 W2 rows for layer 2, by src)
are staged by the host in slot order, so the device only runs sequential
streaming DMA + fp16 matmuls with fp32 PSUM accumulation — no on-device
gather (SWDGE descriptor generation at ~8 ns/row dominates otherwise).

Layer 1 packs slots into 16-dst-node groups padded to 384 slots (3 batches)
and software-pipelines the one-hot phase of tile i+1 against the dense
phase of tile i.  Layer 2 is pure DMA-bandwidth-bound, so its slots are
split main/overflow to cut padding: the first 256 slots of each group go to
the main stream (16-wide 0/1 dst mask, norms pre-folded into the g rows);
group tails go to a per-tile overflow stream with a 128-wide dst mask.
"""

import numpy as np

import concourse.bacc as bacc
import concourse.mybir as mybir
import concourse.tile as tile
from concourse.bass_utils import run_bass_kernel_spmd

F32 = mybir.dt.float32
F16 = mybir.dt.float16

# Problem constants (hardcoded per the harness contract).
N = 32768          # nodes = B*S*NN = 4*16*512
E = 524288         # edges
F_IN, D1, D2 = 128, 256, 128
NT = 3             # edge types
BATCH, SEQ, NNODE = 4, 16, 512
GW = NT * D2       # g row width = 384

NCORES = 8
NPC = N // NCORES          # nodes per core = 4096
GROUP = 16                 # dst nodes per one-hot group
BPG = 3                    # 128-edge batches per group (layer-1 padding)
SLOTS_PG = BPG * 128       # padded edge slots per group = 384
GROUPS_PC = NPC // GROUP   # 256 groups per core
BATCHES_PC = GROUPS_PC * BPG          # 768 batches per core (layer 1)
SLOTS_PC = GROUPS_PC * SLOTS_PG       # 98304 edge slots per core (layer 1)
TILES_PC = NPC // 128      # 32 dst tiles per core
BPT = BPG * 8              # layer-1 batches per dst tile = 24
W_OH = NT * GROUP          # layer-1 one-hot width = 48

# Layer-2 main/overflow split
MAIN_PG = 256                        # main slots per group (2 batches)
MB_PT = (MAIN_PG // 128) * 8         # main batches per tile = 16
MAINB_PC = TILES_PC * MB_PT          # main batches per core = 512
OVF_SLOTS = 384                      # overflow slots per tile (3 batches)
OB_PT = OVF_SLOTS // 128             # overflow batches per tile = 3
OVFB_PC = TILES_PC * OB_PT           # overflow batches per core = 96

# Set by test.py for profiling runs; grading runs keep this off.
TRACE = False
LAST_TIMING = {}

_NC_CACHE = {}


def _build_l1():
    nc = bacc.Bacc("TRN2", target_bir_lowering=False, debug=False)
    # per-slot stream: [xn row (128) | dst-slot id (1) | norms (3)]
    SW = F_IN + 4
    xeoh = nc.dram_tensor(
        "xeoh", [128, BATCHES_PC, SW], F16, kind="ExternalInput")
    iota1 = nc.dram_tensor("iota1", [128, BPT, GROUP], F16, kind="ExternalInput")
    w1 = nc.dram_tensor("w1", [F_IN, NT, D1], F16, kind="ExternalInput")
    b1 = nc.dram_tensor("b1", [128, NT * 2], F32, kind="ExternalInput")
    w2 = nc.dram_tensor("w2", [128, NT, 2, D2], F16, kind="ExternalInput")
    g16 = nc.dram_tensor("g16", [NPC, GW], F16, kind="ExternalOutput")

    with tile.TileContext(nc) as tc:
        with (
            tc.tile_pool(name="const", bufs=1) as cpool,
            tc.tile_pool(name="sb", bufs=4) as sb,
            tc.tile_pool(name="ohp", bufs=3) as ohp,
            tc.tile_pool(name="mt", bufs=4) as mt,
            tc.tile_pool(name="hh", bufs=6) as hh,
            tc.tile_pool(name="sbo", bufs=3) as sbo,
            tc.tile_pool(name="ps", bufs=2, space="PSUM") as ps,
            tc.tile_pool(name="ps2", bufs=3, space="PSUM") as ps2,
            tc.tile_pool(name="ps3", bufs=2, space="PSUM") as ps3,
        ):
            w1_t = cpool.tile([F_IN, NT, D1], F16)
            nc.sync.dma_start(out=w1_t[:], in_=w1[:, :, :])
            b1_t = cpool.tile([128, NT * 2], F32)
            nc.sync.dma_start(out=b1_t[:], in_=b1[:, :])
            w2_t = cpool.tile([128, NT, 2, D2], F16)
            nc.sync.dma_start(out=w2_t[:], in_=w2[:, :, :, :])
            io1_t = cpool.tile([128, BPT, GROUP], F16)
            nc.sync.dma_start(out=io1_t[:], in_=iota1[:, :, :])

            def phase_a(ti):
                """stream + on-DVE one-hot build + aggregation + cast"""
                xg = sb.tile([128, BPT, SW], F16, tag="xg")
                nc.sync.dma_start(
                    out=xg[:], in_=xeoh[:, ti * BPT:(ti + 1) * BPT, :])
                # build the 48-wide norm one-hot from sid + norms
                mk = ohp.tile([128, BPT, GROUP], F16, tag="mk")
                nc.vector.tensor_tensor(
                    mk[:], io1_t[:],
                    xg[:, :, F_IN:F_IN + 1].to_broadcast([128, BPT, GROUP]),
                    mybir.AluOpType.is_equal)
                # [p, t, b, s] so each type's multiply writes contiguously;
                # the matmul rhs reads the (t, s) pair as a 48-wide free dim
                oh_t = ohp.tile([128, NT, BPT, GROUP], F16, tag="oh")
                for t in range(NT):
                    nc.vector.tensor_tensor(
                        oh_t[:, t, :, :], mk[:],
                        xg[:, :, F_IN + 1 + t:F_IN + 2 + t].to_broadcast(
                            [128, BPT, GROUP]),
                        mybir.AluOpType.mult)
                # m1T[f, (group, type, slot)] accumulated per 16-node group
                m1_ps = ps.tile([128, 8 * W_OH], F32, space="PSUM", tag="m1")
                for g8 in range(8):
                    for b in range(BPG):
                        bl = g8 * BPG + b
                        nc.tensor.matmul(
                            out=m1_ps[:, g8 * W_OH:(g8 + 1) * W_OH],
                            lhsT=xg[:, bl, :F_IN],
                            rhs=oh_t[:, :, bl, :],
                            start=(b == 0), stop=(b == BPG - 1),
                        )
                # de-interleave all types: [p, t, (g s)] = [128, 3, 128]
                m1t = mt.tile([128, NT, 128], F16, tag="m1t")
                nc.vector.tensor_copy(
                    out=m1t[:],
                    in_=m1_ps[:].rearrange("p (g t s) -> p t g s", g=8, t=NT))
                return m1t

            def phase_b(ti, m1t):
                """dense h1 = relu(m1 W1 + b1); g = h1 W2; writeback.
                All h1 matmuls are issued before any g matmul so the relus
                complete in the shadow of other PE work."""
                h1ts = []
                for t in range(NT):
                    h1_ps = ps2.tile([128, D1], F32, space="PSUM", tag="h1")
                    h1t = hh.tile([128, D1], F16, tag="h1t")
                    for c in range(2):
                        nc.tensor.matmul(
                            out=h1_ps[:, c * 128:(c + 1) * 128],
                            lhsT=w1_t[:, t, c * 128:(c + 1) * 128],
                            rhs=m1t[:, t, :],
                            start=True, stop=True,
                        )
                        nc.scalar.activation(
                            out=h1t[:, c * 128:(c + 1) * 128],
                            in_=h1_ps[:, c * 128:(c + 1) * 128],
                            func=mybir.ActivationFunctionType.Relu,
                            bias=b1_t[:, t * 2 + c: t * 2 + c + 1], scale=1.0,
                        )
                    h1ts.append(h1t)
                g_sb = sbo.tile([128, GW], F16, tag="gout")
                g_ps = ps3.tile([128, GW], F32, space="PSUM", tag="g")
                for t in range(NT):
                    nc.tensor.matmul(
                        out=g_ps[:, t * D2:(t + 1) * D2],
                        lhsT=h1ts[t][:, :128], rhs=w2_t[:, t, 0, :],
                        start=True, stop=False,
                    )
                    nc.tensor.matmul(
                        out=g_ps[:, t * D2:(t + 1) * D2],
                        lhsT=h1ts[t][:, 128:], rhs=w2_t[:, t, 1, :],
                        start=False, stop=True,
                    )
                nc.vector.tensor_copy(out=g_sb[:], in_=g_ps[:])
                nc.scalar.dma_start(
                    out=g16[ti * 128:(ti + 1) * 128, :], in_=g_sb[:])

            pending = []
            for ti in range(TILES_PC):
                m1t = phase_a(ti)
                pending.append((ti, m1t))
                if len(pending) > 2:
                    phase_b(*pending.pop(0))
            for p in pending:
                phase_b(*p)
    nc.compile()
    return nc


def _build_l2():
    nc = bacc.Bacc("TRN2", target_bir_lowering=False, debug=False)
    # main stream: [norm-scaled g rows (3*128) | dst-slot id (1)]
    gem = nc.dram_tensor(
        "gem", [128, MAINB_PC, GW + 1], F16, kind="ExternalInput")
    # overflow stream: [norm-scaled g rows (3*128) | dst-in-tile id (1)]
    gov = nc.dram_tensor(
        "gov", [128, OVFB_PC, GW + 1], F16, kind="ExternalInput")
    iota2m = nc.dram_tensor(
        "iota2m", [128, MB_PT, GROUP], F16, kind="ExternalInput")
    iota2o = nc.dram_tensor(
        "iota2o", [128, OB_PT, 128], F16, kind="ExternalInput")
    b2 = nc.dram_tensor("b2", [128, NT], F32, kind="ExternalInput")
    out2 = nc.dram_tensor(
        "out2", [D2, TILES_PC, NT, 128], F16, kind="ExternalOutput")

    with tile.TileContext(nc) as tc:
        with (
            tc.tile_pool(name="const", bufs=1) as cpool,
            tc.tile_pool(name="sb", bufs=4) as sb,
            tc.tile_pool(name="mkp", bufs=3) as mkp,
            tc.tile_pool(name="sbo", bufs=3) as sbo,
            tc.tile_pool(name="ps", bufs=4, space="PSUM") as ps,
        ):
            b2_t = cpool.tile([128, NT], F32)
            nc.sync.dma_start(out=b2_t[:], in_=b2[:, :])
            io2m_t = cpool.tile([128, MB_PT, GROUP], F16)
            nc.sync.dma_start(out=io2m_t[:], in_=iota2m[:, :, :])
            io2o_t = cpool.tile([128, OB_PT, 128], F16)
            nc.sync.dma_start(out=io2o_t[:], in_=iota2o[:, :, :])

            for ti in range(TILES_PC):
                gg = sb.tile([128, MB_PT, GW + 1], F16, tag="gg")
                nc.sync.dma_start(
                    out=gg[:], in_=gem[:, ti * MB_PT:(ti + 1) * MB_PT, :])
                go = sb.tile([128, OB_PT, GW + 1], F16, tag="go")
                nc.sync.dma_start(
                    out=go[:], in_=gov[:, ti * OB_PT:(ti + 1) * OB_PT, :])
                mkm = mkp.tile([128, MB_PT, GROUP], F16, tag="mkm")
                nc.vector.tensor_tensor(
                    mkm[:], io2m_t[:],
                    gg[:, :, GW:GW + 1].to_broadcast([128, MB_PT, GROUP]),
                    mybir.AluOpType.is_equal)
                mko = mkp.tile([128, OB_PT, 128], F16, tag="mko")
                nc.vector.tensor_tensor(
                    mko[:], io2o_t[:],
                    go[:, :, GW:GW + 1].to_broadcast([128, OB_PT, 128]),
                    mybir.AluOpType.is_equal)
                o_sb = sbo.tile([128, NT, 128], F16, tag="osb")
                for t in range(NT):
                    # m2T_t [d2, node-within-tile]: main windows + overflow
                    m2_ps = ps.tile([128, 128], F32, space="PSUM", tag="m2")
                    for g8 in range(8):
                        for b in range(2):
                            bl = g8 * 2 + b
                            nc.tensor.matmul(
                                out=m2_ps[:, g8 * GROUP:(g8 + 1) * GROUP],
                                lhsT=gg[:, bl, t * D2:(t + 1) * D2],
                                rhs=mkm[:, bl, :],
                                start=(b == 0), stop=(b == 1),
                            )
                    m2o_ps = ps.tile([128, 128], F32, space="PSUM", tag="m2o")
                    for b in range(OB_PT):
                        nc.tensor.matmul(
                            out=m2o_ps[:],
                            lhsT=go[:, b, t * D2:(t + 1) * D2],
                            rhs=mko[:, b, :],
                            start=(b == 0), stop=(b == OB_PT - 1),
                        )
                    o2_sb = sbo.tile([128, 128], F32, tag="o2sb")
                    nc.vector.tensor_copy(out=o2_sb[:], in_=m2o_ps[:])
                    s_sb = sbo.tile([128, 128], F32, tag="ssb")
                    nc.vector.tensor_tensor(
                        s_sb[:], m2_ps[:], o2_sb[:], mybir.AluOpType.add)
                    nc.scalar.activation(
                        out=o_sb[:, t, :], in_=s_sb[:],
                        func=mybir.ActivationFunctionType.Relu,
                        bias=b2_t[:, t:t + 1], scale=1.0,
                    )
                nc.scalar.dma_start(out=out2[:, ti, :, :], in_=o_sb[:])
    nc.compile()
    return nc


def _host_prep(x, edge_attr, edge_index):
    """Sort/shard/pad edges, normalize x, stage the layer-1 stream and the
    layer-2 slot assignment (main/overflow)."""
    src = np.asarray(edge_index[0], np.int64)
    dst = np.asarray(edge_index[1], np.int64)
    ew = np.abs(np.asarray(edge_attr, np.float32))          # [E, 3]

    deg = np.empty((N, NT), np.float32)
    for t in range(NT):
        deg[:, t] = np.bincount(dst, weights=ew[:, t], minlength=N)
    deg += 1.0
    dis = 1.0 / np.sqrt(deg)

    norm = dis[src] * ew * dis[dst]                          # [E, 3]
    src_all = np.concatenate([src, np.arange(N)])
    dst_all = np.concatenate([dst, np.arange(N)])
    norm_all = np.concatenate([norm, 1.0 / deg]).astype(np.float32)

    order = np.argsort(dst_all, kind="stable")
    sa = src_all[order]
    da = dst_all[order]
    na = norm_all[order].astype(np.float16)

    gid = da >> 4                                            # 16-node group id
    counts = np.bincount(gid, minlength=N // GROUP)
    assert counts.max() <= SLOTS_PG, (
        f"group overflow: {counts.max()} > {SLOTS_PG}")
    gstart = np.zeros(N // GROUP + 1, np.int64)
    np.cumsum(counts, out=gstart[1:])
    rank = np.arange(da.size) - gstart[gid]

    # ---- layer-1 slot layout: 384 padded slots per group -------------
    pos = gid * SLOTS_PG + rank
    n_slots = (N // GROUP) * SLOTS_PG
    src_pad = np.zeros(n_slots, np.int64)
    src_pad[pos] = sa
    slot = (da & (GROUP - 1)).astype(np.int64)
    sid1 = np.full(n_slots, 255.0, np.float16)
    sid1[pos] = slot
    na1 = np.zeros((n_slots, NT), np.float16)
    na1[pos] = na

    # ---- layer-2 slot layout: 256 main slots per group + overflow ----
    mm = rank < MAIN_PG
    pos_m = gid[mm] * MAIN_PG + rank[mm]
    n_main = (N // GROUP) * MAIN_PG
    src_m = np.zeros(n_main, np.int64)
    src_m[pos_m] = sa[mm]
    na_m = np.zeros((n_main, NT), np.float16)
    na_m[pos_m] = na[mm]
    sid_m = np.full(n_main, 255.0, np.float16)
    sid_m[pos_m] = slot[mm]

    ov = ~mm
    tile_e = da[ov] >> 7                                     # global dst tile
    cnt_o = np.bincount(tile_e, minlength=N // 128)
    assert cnt_o.max() <= OVF_SLOTS, (
        f"tile overflow: {cnt_o.max()} > {OVF_SLOTS}")
    st_o = np.zeros(N // 128 + 1, np.int64)
    np.cumsum(cnt_o, out=st_o[1:])
    r2 = np.arange(tile_e.size) - st_o[tile_e]
    pos_o = tile_e * OVF_SLOTS + r2
    n_ovf = (N // 128) * OVF_SLOTS
    src_o = np.zeros(n_ovf, np.int64)
    src_o[pos_o] = sa[ov]
    na_o = np.zeros((n_ovf, NT), np.float16)
    na_o[pos_o] = na[ov]
    sid_o = np.full(n_ovf, 255.0, np.float16)
    sid_o[pos_o] = da[ov] & 127

    # normalize x on the host (fp16 device math, fp32 accumulation)
    mu = x.mean(axis=0)
    sg = x.std(axis=0, ddof=1)
    xn16 = ((x - mu[None, :]) / sg[None, :]).astype(np.float16)

    def pb(a, nb):
        """[nb*128, ...] -> [128, nb, ...] (partition = slot % 128)"""
        return a.reshape((nb, 128) + a.shape[1:]).swapaxes(0, 1)

    per_core = []
    for k in range(NCORES):
        # [p, b] layout everywhere: partition = slot % 128, batch = slot // 128
        s1 = slice(k * SLOTS_PC, (k + 1) * SLOTS_PC)
        idx1 = pb(src_pad[s1], BATCHES_PC)
        xeoh = np.empty((128, BATCHES_PC, F_IN + 4), np.float16)
        np.take(xn16, idx1, axis=0, out=xeoh[:, :, :F_IN])
        xeoh[:, :, F_IN] = pb(sid1[s1], BATCHES_PC)
        xeoh[:, :, F_IN + 1:] = pb(na1[s1], BATCHES_PC)

        s_m = slice(k * MAINB_PC * 128, (k + 1) * MAINB_PC * 128)
        s_o = slice(k * OVFB_PC * 128, (k + 1) * OVFB_PC * 128)
        per_core.append((
            xeoh,
            pb(src_m[s_m], MAINB_PC), pb(na_m[s_m], MAINB_PC),
            pb(sid_m[s_m], MAINB_PC),
            pb(src_o[s_o], OVFB_PC), pb(na_o[s_o], OVFB_PC),
            pb(sid_o[s_o], OVFB_PC),
        ))
    return per_core


def _stage_l2(g_full, idx_pb, na_pb, sid_pb, nb):
    """Build a layer-2 stream tensor [128, nb, GW + 1]: norm-scaled
    gathered g rows followed by the dst slot id."""
    out = np.empty((128, nb, GW + 1), np.float16)
    np.take(g_full, idx_pb, axis=0, out=out[:, :, :GW])
    for t in range(NT):
        out[:, :, t * D2:(t + 1) * D2] *= na_pb[:, :, t:t + 1]
    out[:, :, GW] = sid_pb
    return out


def kernel(x, edge_attr, W1, b1, W2, b2, edge_index, batch_size, seq_len,
           n_nodes):
    x = np.asarray(x, np.float32)
    edge_attr = np.asarray(edge_attr, np.float32)
    W1 = np.asarray(W1, np.float32)
    b1 = np.asarray(b1, np.float32)
    W2 = np.asarray(W2, np.float32)
    b2 = np.asarray(b2, np.float32)
    edge_index = np.asarray(edge_index)
    assert x.shape == (N, F_IN) and edge_index.shape == (2, E)

    per_core = _host_prep(x, edge_attr, edge_index)

    # ---- launch 1 ----
    if "l1" not in _NC_CACHE:
        _NC_CACHE["l1"] = _build_l1()
    nc1 = _NC_CACHE["l1"]

    w1_in = np.ascontiguousarray(W1.transpose(1, 0, 2)).astype(np.float16)
    b1_in = np.ascontiguousarray(
        b1.reshape(NT, 2, 128).transpose(2, 0, 1).reshape(128, NT * 2))
    w2_in = np.ascontiguousarray(
        W2.reshape(NT, 2, 128, D2).transpose(2, 0, 1, 3)).astype(np.float16)

    iota1_in = np.broadcast_to(
        np.arange(GROUP, dtype=np.float16), (128, BPT, GROUP)).copy()
    in_maps1 = []
    for k in range(NCORES):
        in_maps1.append({
            "xeoh": per_core[k][0], "iota1": iota1_in,
            "w1": w1_in, "b1": b1_in, "w2": w2_in,
        })
    res1 = run_bass_kernel_spmd(
        nc1, in_maps1, core_ids=list(range(NCORES)), trace=TRACE)
    if TRACE:
        LAST_TIMING["l1_ns"] = res1.exec_time_ns

    g_full = np.concatenate(
        [res1.results[k]["g16"] for k in range(NCORES)], axis=0)  # [N, 384] f16

    # ---- launch 2 ----
    if "l2" not in _NC_CACHE:
        _NC_CACHE["l2"] = _build_l2()
    nc2 = _NC_CACHE["l2"]

    b2_in = np.ascontiguousarray(b2.T)                            # [128, 3]
    iota2m_in = np.broadcast_to(
        np.arange(GROUP, dtype=np.float16), (128, MB_PT, GROUP)).copy()
    iota2o_in = np.broadcast_to(
        np.arange(128, dtype=np.float16), (128, OB_PT, 128)).copy()
    in_maps2 = []
    for k in range(NCORES):
        _, idx_m, na_m_pb, sid_m_pb, idx_o, na_o_pb, sid_o_pb = per_core[k]
        in_maps2.append({
            "gem": _stage_l2(g_full, idx_m, na_m_pb, sid_m_pb, MAINB_PC),
            "gov": _stage_l2(g_full, idx_o, na_o_pb, sid_o_pb, OVFB_PC),
            "iota2m": iota2m_in, "iota2o": iota2o_in,
            "b2": b2_in,
        })
    res2 = run_bass_kernel_spmd(
        nc2, in_maps2, core_ids=list(range(NCORES)), trace=TRACE)
    if TRACE:
        LAST_TIMING["l2_ns"] = res2.exec_time_ns

    # per-core out2 [D2, TILES, NT, 128] -> [NT, D2, NPC]; concat cores
    m2t = np.concatenate(
        [res2.results[k]["out2"].transpose(2, 0, 1, 3).reshape(NT, D2, NPC)
         for k in range(NCORES)], axis=2)                          # [3,128,N] f16

    # [3, 128, (b, s, nn)] -> out[(b, nn), s, (t, d)]
    out = m2t.astype(np.float32).reshape(NT, D2, BATCH, SEQ, NNODE)
    out = out.transpose(2, 4, 3, 0, 1)
    out = np.ascontiguousarray(
        out.reshape(BATCH * NNODE, SEQ, NT * D2), dtype=np.float32)
    return out
